# revision 12
# baseline (speedup 1.0000x reference)
"""Bidirectional LSTM encoder on 8 Trainium2 NeuronCores.

Sharding: pure data parallel. Core c = (direction d = c//4, batch quarter
q = c%4, 16 rows each). Each core runs the same SPMD program: embedding
gather, x@Wx precompute fused into the per-step gate accumulation, and the
512-step LSTM recurrence for its 16-row batch slice. Backward-direction
cores receive the time-reversed token sequence; the host reverses their
outputs back. No cross-core communication.

Per-step PE layout: gates [16, 4096] live in PSUM as four 16-row blocks at
partition bases 0/32/64/96 (one per gate i/f/g/o) so four matmul streams
run concurrently on distinct 32-column groups of the PE array. Contraction
is over E (precomputed xsT tiles) + H (hT state tiles), both as [128, 16]
bf16 stationaries with N=512 bf16 moving operands from the weight tiles.
ACT applies sigmoid/tanh straight out of PSUM; DVE does the c/h updates in
bf16; PE transpose mode regenerates hT for the next step.
"""

import sys

sys.path.insert(0, "/opt/trn_rl_repo")

import numpy as np
import ml_dtypes

import concourse.bacc as bacc
import concourse.mybir as mybir
import concourse.tile as tile
from concourse import bass_utils

BF16 = mybir.dt.bfloat16
F32 = mybir.dt.float32
I32 = mybir.dt.int32

AF = mybir.ActivationFunctionType
OP = mybir.AluOpType

E = 1024
H = 1024
G = 4096          # 4*H
BP = 16           # batch rows per core
KE = E // 128     # 8 contraction tiles over E
KH = H // 128     # 8 contraction tiles over H
RING = 6          # xsT lookahead ring (m-tiles)
PREP_AHEAD = 5    # m-tiles prepped ahead of consumption


def build_program(S=512, V=32000, num_devices=8):
    """Build + compile the per-core SPMD program. Returns the Bacc module."""
    nc = bacc.Bacc("TRN2", target_bir_lowering=False, debug=False,
                   num_devices=num_devices)

    NM = S // 8  # number of 128-row m-tiles (8 steps x 16 batch)

    tok_d = nc.dram_tensor("tok", [S * BP, 1], I32, kind="ExternalInput").ap()
    emb_d = nc.dram_tensor("emb", [V, E], BF16, kind="ExternalInput").ap()
    wx_d = nc.dram_tensor("wx", [E, G], BF16, kind="ExternalInput").ap()
    wh_d = nc.dram_tensor("wh", [H, G], BF16, kind="ExternalInput").ap()
    bias_d = nc.dram_tensor("bias", [4, H], BF16, kind="ExternalInput").ap()
    eye16_d = nc.dram_tensor("eye16", [16, 16], BF16, kind="ExternalInput").ap()
    eye128_d = nc.dram_tensor("eye128", [128, 128], BF16, kind="ExternalInput").ap()
    sel4_d = nc.dram_tensor("sel4", [4, 128], BF16, kind="ExternalInput").ap()

    hs_d = nc.dram_tensor("hs", [S, BP, H], BF16, kind="ExternalOutput").ap()
    cout_d = nc.dram_tensor("c_out", [BP, H], F32, kind="ExternalOutput").ap()

    with tile.TileContext(nc) as tc:
        with (
            tc.tile_pool(name="wpool", bufs=1) as wpool,
            tc.tile_pool(name="xst", bufs=RING) as xst_pool,
            tc.tile_pool(name="embp", bufs=3) as emb_pool,
            tc.tile_pool(name="tokp", bufs=2) as tok_pool,
            tc.tile_pool(name="state", bufs=2) as state_pool,
            tc.tile_pool(name="ew", bufs=3) as ew_pool,
            tc.tile_pool(name="hp", bufs=3) as h_pool,
            tc.tile_pool(name="psg", bufs=3, space="PSUM") as psg_pool,
            tc.tile_pool(name="pstx", bufs=2, space="PSUM") as pstx_pool,
        ):
            # ---- persistent weights / constants ----
            wx_sb = wpool.tile([128, KE * G], BF16, tag="wx")
            wh_sb = wpool.tile([128, KH * G], BF16, tag="wh")
            bias_sb = wpool.tile([4, H], BF16, tag="bias")
            eye16 = wpool.tile([16, 16], BF16, tag="eye16")
            eye128 = wpool.tile([128, 128], BF16, tag="eye128")
            sel4 = wpool.tile([4, 128], BF16, tag="sel4")

            wx_r = wx_d.rearrange("(k p) n -> k p n", p=128)
            wh_r = wh_d.rearrange("(k p) n -> k p n", p=128)
            for k in range(KE):
                nc.sync.dma_start(out=wx_sb[:, k * G:(k + 1) * G], in_=wx_r[k])
            for k in range(KH):
                nc.sync.dma_start(out=wh_sb[:, k * G:(k + 1) * G], in_=wh_r[k])
            nc.sync.dma_start(out=bias_sb[:], in_=bias_d[:])
            nc.sync.dma_start(out=eye16[:], in_=eye16_d[:])
            nc.sync.dma_start(out=eye128[:], in_=eye128_d[:])
            nc.sync.dma_start(out=sel4[:], in_=sel4_d[:])

            xst_tiles = {}
            psum_tiles = {}

            def mtile_prep(j):
                tok_t = tok_pool.tile([128, 1], I32, tag="tok")
                gr = emb_pool.tile([128, E], BF16, tag="emb")
                nc.sync.dma_start(out=tok_t[:], in_=tok_d[128 * j:128 * (j + 1), :])
                import concourse.bass as bass
                nc.gpsimd.indirect_dma_start(
                    out=gr[:], out_offset=None, in_=emb_d[:],
                    in_offset=bass.IndirectOffsetOnAxis(ap=tok_t[:, :1], axis=0),
                )
                xst = xst_pool.tile([128, E], BF16, tag="xst")
                xst_tiles[j] = xst
                for e in range(KE):
                    psx = pstx_pool.tile([128, 128], BF16, tag="pstx")
                    nc.tensor.transpose(out=psx[:], in_=gr[:, 128 * e:128 * (e + 1)],
                                        identity=eye128[:])
                    nc.scalar.copy(out=xst[:, 128 * e:128 * (e + 1)], in_=psx[:])

            def x_mms(t):
                ps = psg_pool.tile([128, 1024], F32, tag="psg")
                psum_tiles[t] = ps
                j, lt = t // 8, t % 8
                xst = xst_tiles[j]
                for ch in range(2):
                    nc.tensor.matmul(
                        out=ps[:, 512 * ch:512 * (ch + 1)],
                        lhsT=sel4[:], rhs=bias_sb[:, 512 * ch:512 * (ch + 1)],
                        start=True, stop=False, skip_group_check=True,
                        tile_position=(0, 0))
                for e in range(KE):
                    lhsT = xst[:, 128 * e + 16 * lt:128 * e + 16 * lt + 16]
                    for g in range(4):
                        for ch in range(2):
                            nc.tensor.matmul(
                                out=ps[32 * g:32 * g + BP, 512 * ch:512 * (ch + 1)],
                                lhsT=lhsT,
                                rhs=wx_sb[:, G * e + 1024 * g + 512 * ch:][:, :512],
                                start=False, stop=False, skip_group_check=True,
                                tile_position=(0, 32 * g))

            def h_mms(t, hT_cur):
                ps = psum_tiles[t]
                for k in range(KH):
                    lhsT = hT_cur[:, 16 * k:16 * (k + 1)]
                    for g in range(4):
                        for ch in range(2):
                            nc.tensor.matmul(
                                out=ps[32 * g:32 * g + BP, 512 * ch:512 * (ch + 1)],
                                lhsT=lhsT,
                                rhs=wh_sb[:, G * k + 1024 * g + 512 * ch:][:, :512],
                                start=False, stop=False, skip_group_check=True,
                                tile_position=(0, 32 * g))

            # ---- prologue ----
            hT_cur = state_pool.tile([128, 128], BF16, tag="hT")
            c_cur = state_pool.tile([BP, H], BF16, tag="c")
            nc.gpsimd.memset(hT_cur[:], 0.0)
            nc.gpsimd.memset(c_cur[:], 0.0)

            for j in range(min(PREP_AHEAD, NM)):
                mtile_prep(j)
            x_mms(0)
            if S > 1:
                x_mms(1)

            # ---- main loop ----
            for t in range(S):
                if t % 8 == 0 and t // 8 + PREP_AHEAD < NM:
                    mtile_prep(t // 8 + PREP_AHEAD)

                h_mms(t, hT_cur)
                ps = psum_tiles.pop(t)
                # emit next-next step's x accumulation now so the PE has
                # independent work while ACT/DVE run this step's gate math
                if t + 2 < S:
                    x_mms(t + 2)

                c_next = state_pool.tile([BP, H], BF16, tag="c")
                h_t = h_pool.tile([BP, H], BF16, tag="h")
                hT_next = state_pool.tile([128, 128], BF16, tag="hT")

                for q in range(2):
                    sl = slice(512 * q, 512 * (q + 1))
                    sf = ew_pool.tile([BP, 512], BF16, tag="sf")
                    si = ew_pool.tile([BP, 512], BF16, tag="si")
                    tg = ew_pool.tile([BP, 512], BF16, tag="tg")
                    so = ew_pool.tile([BP, 512], BF16, tag="so")
                    t1 = ew_pool.tile([BP, 512], BF16, tag="t1")
                    u = ew_pool.tile([BP, 512], BF16, tag="u")
                    tc_t = ew_pool.tile([BP, 512], BF16, tag="tc")
                    nc.scalar.activation(out=sf[:], in_=ps[32:32 + BP, sl], func=AF.Sigmoid)
                    nc.scalar.activation(out=si[:], in_=ps[0:BP, sl], func=AF.Sigmoid)
                    nc.scalar.activation(out=tg[:], in_=ps[64:64 + BP, sl], func=AF.Tanh)
                    nc.vector.tensor_tensor(out=t1[:], in0=sf[:], in1=c_cur[:, sl], op=OP.mult)
                    nc.vector.tensor_tensor(out=u[:], in0=si[:], in1=tg[:], op=OP.mult)
                    nc.scalar.activation(out=so[:], in_=ps[96:96 + BP, sl], func=AF.Sigmoid)
                    nc.vector.tensor_tensor(out=c_next[:, sl], in0=t1[:], in1=u[:], op=OP.add)
                    nc.scalar.activation(out=tc_t[:], in_=c_next[:, sl], func=AF.Tanh)
                    nc.vector.tensor_tensor(out=h_t[:, sl], in0=so[:], in1=tc_t[:], op=OP.mult)
                    # transpose the finished h half into next-step stationary
                    for k in range(4 * q, 4 * q + 4):
                        pst = pstx_pool.tile([128, 16], BF16, tag="pstx")
                        nc.tensor.transpose(out=pst[:], in_=h_t[:, 128 * k:128 * (k + 1)],
                                            identity=eye16[:])
                        nc.vector.tensor_copy(out=hT_next[:, 16 * k:16 * (k + 1)], in_=pst[:])

                nc.sync.dma_start(out=hs_d[t], in_=h_t[:])

                hT_cur = hT_next
                c_cur = c_next

            nc.gpsimd.dma_start(out=cout_d[:], in_=c_cur[:])  # bf16 -> f32 cast

    nc.compile()
    return nc


_PROGRAM_CACHE = {}


def _get_program(S=512, V=32000):
    key = (S, V)
    if key not in _PROGRAM_CACHE:
        _PROGRAM_CACHE[key] = build_program(S=S, V=V)
    return _PROGRAM_CACHE[key]


def make_in_maps(inputs, embed_table, Wx_f, Wh_f, b_f, Wx_b, Wh_b, b_b):
    """Build the 8 per-core input dicts from full-size inputs."""
    bf = ml_dtypes.bfloat16
    B, S = inputs.shape
    emb = np.ascontiguousarray(embed_table, dtype=bf)
    eye16 = np.eye(16, dtype=bf)
    eye128 = np.eye(128, dtype=bf)
    sel4 = np.repeat(np.eye(4, dtype=bf), 32, axis=1)  # [4, 128]
    per_dir = {
        0: (np.ascontiguousarray(Wx_f, dtype=bf), np.ascontiguousarray(Wh_f, dtype=bf),
            np.ascontiguousarray(b_f.reshape(4, -1), dtype=bf)),
        1: (np.ascontiguousarray(Wx_b, dtype=bf), np.ascontiguousarray(Wh_b, dtype=bf),
            np.ascontiguousarray(b_b.reshape(4, -1), dtype=bf)),
    }
    in_maps = []
    for c in range(8):
        d, q = c // 4, c % 4
        tok = inputs[BP * q:BP * (q + 1), :]          # [16, S]
        if d == 1:
            tok = tok[:, ::-1]
        tok = np.ascontiguousarray(tok.T.reshape(-1, 1), dtype=np.int32)  # [(t,b), 1]
        wx, wh, bias = per_dir[d]
        in_maps.append({
            "tok": tok, "emb": emb, "wx": wx, "wh": wh, "bias": bias,
            "eye16": eye16, "eye128": eye128, "sel4": sel4,
        })
    return in_maps


def run_cores(in_maps, S=512, V=32000, trace=False, tmpdir=None):
    nc = _get_program(S=S, V=V)
    kwargs = {}
    if trace:
        kwargs = dict(trace=True, tmpdir=tmpdir)
    res = bass_utils.run_bass_kernel_spmd(nc, in_maps, core_ids=list(range(8)), **kwargs)
    return res


def assemble(results, B=64, S=512):
    outputs = np.empty((B, S, 2 * H), dtype=np.float32)
    hidden = np.empty((B, 2 * H), dtype=np.float32)
    cell = np.empty((B, 2 * H), dtype=np.float32)
    for c in range(8):
        d, q = c // 4, c % 4
        hs = np.asarray(results[c]["hs"], dtype=np.float32)   # [S, 16, H]
        c_out = np.asarray(results[c]["c_out"], dtype=np.float32)
        rows = slice(BP * q, BP * (q + 1))
        if d == 0:
            outputs[rows, :, :H] = hs.transpose(1, 0, 2)
            hidden[rows, :H] = hs[-1]
            cell[rows, :H] = c_out
        else:
            outputs[rows, :, H:] = hs[::-1].transpose(1, 0, 2)
            hidden[rows, H:] = hs[-1]
            cell[rows, H:] = c_out
    return outputs, hidden, cell


def kernel(inputs, embed_table, Wx_f, Wh_f, b_f, Wx_b, Wh_b, b_b):
    B, S = inputs.shape
    in_maps = make_in_maps(inputs, embed_table, Wx_f, Wh_f, b_f, Wx_b, Wh_b, b_b)
    res = run_cores(in_maps, S=S, V=embed_table.shape[0])
    return assemble(res.results, B=B, S=S)


# revision 14
# speedup vs baseline: 1.0413x; 1.0413x over previous
"""Bidirectional LSTM encoder on 8 Trainium2 NeuronCores.

Sharding: pure data parallel. Core c = (direction d = c//4, batch quarter
q = c%4, 16 rows each). Each core runs the same SPMD program: embedding
gather, x@Wx precompute fused into the per-step gate accumulation, and the
512-step LSTM recurrence for its 16-row batch slice. Backward-direction
cores receive the time-reversed token sequence; the host reverses their
outputs back. No cross-core communication.

Per-step PE layout: gates [16, 4096] live in PSUM as four 16-row blocks at
partition bases 0/32/64/96 (one per gate i/f/g/o) so four matmul streams
run concurrently on distinct 32-column groups of the PE array. Contraction
is over E (precomputed xsT tiles) + H (hT state tiles), both as [128, 16]
bf16 stationaries with N=512 bf16 moving operands from the weight tiles.
ACT applies sigmoid/tanh straight out of PSUM; DVE does the c/h updates in
bf16; PE transpose mode regenerates hT for the next step.
"""

import sys

sys.path.insert(0, "/opt/trn_rl_repo")

import numpy as np
import ml_dtypes

import concourse.bacc as bacc
import concourse.mybir as mybir
import concourse.tile as tile
from concourse import bass_utils

BF16 = mybir.dt.bfloat16
F32 = mybir.dt.float32
I32 = mybir.dt.int32

AF = mybir.ActivationFunctionType
OP = mybir.AluOpType

E = 1024
H = 1024
G = 4096          # 4*H
BP = 16           # batch rows per core
KE = E // 128     # 8 contraction tiles over E
KH = H // 128     # 8 contraction tiles over H
RING = 6          # xsT lookahead ring (m-tiles)
PREP_AHEAD = 5    # m-tiles prepped ahead of consumption


def build_program(S=512, V=32000, num_devices=8):
    """Build + compile the per-core SPMD program. Returns the Bacc module."""
    nc = bacc.Bacc("TRN2", target_bir_lowering=False, debug=False,
                   num_devices=num_devices)

    NM = S // 8  # number of 128-row m-tiles (8 steps x 16 batch)

    tok_d = nc.dram_tensor("tok", [S * BP, 1], I32, kind="ExternalInput").ap()
    emb_d = nc.dram_tensor("emb", [V, E], BF16, kind="ExternalInput").ap()
    wx_d = nc.dram_tensor("wx", [E, G], BF16, kind="ExternalInput").ap()
    wh_d = nc.dram_tensor("wh", [H, G], BF16, kind="ExternalInput").ap()
    bias_d = nc.dram_tensor("bias", [4, H], BF16, kind="ExternalInput").ap()
    eye16_d = nc.dram_tensor("eye16", [16, 16], BF16, kind="ExternalInput").ap()
    eye128_d = nc.dram_tensor("eye128", [128, 128], BF16, kind="ExternalInput").ap()
    sel4_d = nc.dram_tensor("sel4", [4, 128], BF16, kind="ExternalInput").ap()

    hs_d = nc.dram_tensor("hs", [S, BP, H], BF16, kind="ExternalOutput").ap()
    cout_d = nc.dram_tensor("c_out", [BP, H], F32, kind="ExternalOutput").ap()

    with tile.TileContext(nc) as tc:
        with (
            tc.tile_pool(name="wpool", bufs=1) as wpool,
            tc.tile_pool(name="xst", bufs=RING) as xst_pool,
            tc.tile_pool(name="embp", bufs=3) as emb_pool,
            tc.tile_pool(name="tokp", bufs=2) as tok_pool,
            tc.tile_pool(name="state", bufs=2) as state_pool,
            tc.tile_pool(name="ew", bufs=3) as ew_pool,
            tc.tile_pool(name="hp", bufs=3) as h_pool,
            tc.tile_pool(name="psg", bufs=3, space="PSUM") as psg_pool,
            tc.tile_pool(name="pstx", bufs=2, space="PSUM") as pstx_pool,
        ):
            # ---- persistent weights / constants ----
            wx_sb = wpool.tile([128, KE * G], BF16, tag="wx")
            wh_sb = wpool.tile([128, KH * G], BF16, tag="wh")
            bias_sb = wpool.tile([4, H], BF16, tag="bias")
            eye16 = wpool.tile([16, 16], BF16, tag="eye16")
            eye128 = wpool.tile([128, 128], BF16, tag="eye128")
            sel4 = wpool.tile([4, 128], BF16, tag="sel4")

            wx_r = wx_d.rearrange("(k p) n -> k p n", p=128)
            wh_r = wh_d.rearrange("(k p) n -> k p n", p=128)
            for k in range(KE):
                nc.sync.dma_start(out=wx_sb[:, k * G:(k + 1) * G], in_=wx_r[k])
            for k in range(KH):
                nc.sync.dma_start(out=wh_sb[:, k * G:(k + 1) * G], in_=wh_r[k])
            nc.sync.dma_start(out=bias_sb[:], in_=bias_d[:])
            nc.sync.dma_start(out=eye16[:], in_=eye16_d[:])
            nc.sync.dma_start(out=eye128[:], in_=eye128_d[:])
            nc.sync.dma_start(out=sel4[:], in_=sel4_d[:])

            xst_tiles = {}
            psum_tiles = {}

            def mtile_prep(j):
                tok_t = tok_pool.tile([128, 1], I32, tag="tok")
                gr = emb_pool.tile([128, E], BF16, tag="emb")
                nc.sync.dma_start(out=tok_t[:], in_=tok_d[128 * j:128 * (j + 1), :])
                import concourse.bass as bass
                nc.gpsimd.indirect_dma_start(
                    out=gr[:], out_offset=None, in_=emb_d[:],
                    in_offset=bass.IndirectOffsetOnAxis(ap=tok_t[:, :1], axis=0),
                )
                xst = xst_pool.tile([128, E], BF16, tag="xst")
                xst_tiles[j] = xst
                for e in range(KE):
                    psx = pstx_pool.tile([128, 128], BF16, tag="pstx")
                    nc.tensor.transpose(out=psx[:], in_=gr[:, 128 * e:128 * (e + 1)],
                                        identity=eye128[:])
                    nc.scalar.copy(out=xst[:, 128 * e:128 * (e + 1)], in_=psx[:])

            def x_mms(t):
                ps = psg_pool.tile([128, 1024], F32, tag="psg")
                psum_tiles[t] = ps
                j, lt = t // 8, t % 8
                xst = xst_tiles[j]
                for ch in range(2):
                    nc.tensor.matmul(
                        out=ps[:, 512 * ch:512 * (ch + 1)],
                        lhsT=sel4[:], rhs=bias_sb[:, 512 * ch:512 * (ch + 1)],
                        start=True, stop=False, skip_group_check=True,
                        tile_position=(0, 0))
                for e in range(KE):
                    lhsT = xst[:, 128 * e + 16 * lt:128 * e + 16 * lt + 16]
                    for g in range(4):
                        for ch in range(2):
                            nc.tensor.matmul(
                                out=ps[32 * g:32 * g + BP, 512 * ch:512 * (ch + 1)],
                                lhsT=lhsT,
                                rhs=wx_sb[:, G * e + 1024 * g + 512 * ch:][:, :512],
                                start=False, stop=False, skip_group_check=True,
                                tile_position=(0, 32 * g))

            def h_mms_part(t, hT_t, k0, k1):
                ps = psum_tiles[t]
                for k in range(k0, k1):
                    lhsT = hT_t[:, 16 * k:16 * (k + 1)]
                    for g in range(4):
                        for ch in range(2):
                            nc.tensor.matmul(
                                out=ps[32 * g:32 * g + BP, 512 * ch:512 * (ch + 1)],
                                lhsT=lhsT,
                                rhs=wh_sb[:, G * k + 1024 * g + 512 * ch:][:, :512],
                                start=False, stop=False, skip_group_check=True,
                                tile_position=(0, 32 * g))

            # ---- prologue ----
            hT_cur = state_pool.tile([128, 128], BF16, tag="hT")
            c_cur = state_pool.tile([BP, H], BF16, tag="c")
            nc.gpsimd.memset(hT_cur[:], 0.0)
            nc.gpsimd.memset(c_cur[:], 0.0)

            def ew_half(ps, c_cur, c_next, h_t, q):
                sl = slice(512 * q, 512 * (q + 1))
                sf = ew_pool.tile([BP, 512], BF16, tag="sf")
                si = ew_pool.tile([BP, 512], BF16, tag="si")
                tg = ew_pool.tile([BP, 512], BF16, tag="tg")
                so = ew_pool.tile([BP, 512], BF16, tag="so")
                t1 = ew_pool.tile([BP, 512], BF16, tag="t1")
                u = ew_pool.tile([BP, 512], BF16, tag="u")
                tc_t = ew_pool.tile([BP, 512], BF16, tag="tc")
                nc.scalar.activation(out=sf[:], in_=ps[32:32 + BP, sl], func=AF.Sigmoid)
                nc.scalar.activation(out=si[:], in_=ps[0:BP, sl], func=AF.Sigmoid)
                nc.scalar.activation(out=tg[:], in_=ps[64:64 + BP, sl], func=AF.Tanh)
                nc.vector.tensor_tensor(out=t1[:], in0=sf[:], in1=c_cur[:, sl], op=OP.mult)
                nc.vector.tensor_tensor(out=u[:], in0=si[:], in1=tg[:], op=OP.mult)
                nc.scalar.activation(out=so[:], in_=ps[96:96 + BP, sl], func=AF.Sigmoid)
                nc.vector.tensor_tensor(out=c_next[:, sl], in0=t1[:], in1=u[:], op=OP.add)
                nc.scalar.activation(out=tc_t[:], in_=c_next[:, sl], func=AF.Tanh)
                nc.vector.tensor_tensor(out=h_t[:, sl], in0=so[:], in1=tc_t[:], op=OP.mult)

            def trans_half(h_t, hT_next, q):
                for k in range(4 * q, 4 * q + 4):
                    pst = pstx_pool.tile([128, 16], BF16, tag="pstx")
                    nc.tensor.transpose(out=pst[:], in_=h_t[:, 128 * k:128 * (k + 1)],
                                        identity=eye16[:])
                    nc.vector.tensor_copy(out=hT_next[:, 16 * k:16 * (k + 1)], in_=pst[:])

            for j in range(min(PREP_AHEAD, NM)):
                mtile_prep(j)
            x_mms(0)
            if S > 1:
                x_mms(1)
            # first half of step 0's recurrent matmuls (h0 = 0)
            h_mms_part(0, hT_cur, 0, 4)

            # ---- main loop (software-pipelined at half-H granularity) ----
            for t in range(S):
                if t % 8 == 0 and t // 8 + PREP_AHEAD < NM:
                    mtile_prep(t // 8 + PREP_AHEAD)

                h_mms_part(t, hT_cur, 4, KH)
                ps = psum_tiles.pop(t)
                # independent PE work to cover this step's ACT/DVE chain
                if t + 2 < S:
                    x_mms(t + 2)

                c_next = state_pool.tile([BP, H], BF16, tag="c")
                h_t = h_pool.tile([BP, H], BF16, tag="h")

                ew_half(ps, c_cur, c_next, h_t, 0)
                if t + 1 < S:
                    hT_next = state_pool.tile([128, 128], BF16, tag="hT")
                    trans_half(h_t, hT_next, 0)
                    h_mms_part(t + 1, hT_next, 0, 4)
                ew_half(ps, c_cur, c_next, h_t, 1)
                if t + 1 < S:
                    trans_half(h_t, hT_next, 1)
                    hT_cur = hT_next

                nc.sync.dma_start(out=hs_d[t], in_=h_t[:])
                c_cur = c_next

            nc.gpsimd.dma_start(out=cout_d[:], in_=c_cur[:])  # bf16 -> f32 cast

    nc.compile()
    return nc


_PROGRAM_CACHE = {}


def _get_program(S=512, V=32000):
    key = (S, V)
    if key not in _PROGRAM_CACHE:
        _PROGRAM_CACHE[key] = build_program(S=S, V=V)
    return _PROGRAM_CACHE[key]


def make_in_maps(inputs, embed_table, Wx_f, Wh_f, b_f, Wx_b, Wh_b, b_b):
    """Build the 8 per-core input dicts from full-size inputs."""
    bf = ml_dtypes.bfloat16
    B, S = inputs.shape
    emb = np.ascontiguousarray(embed_table, dtype=bf)
    eye16 = np.eye(16, dtype=bf)
    eye128 = np.eye(128, dtype=bf)
    sel4 = np.repeat(np.eye(4, dtype=bf), 32, axis=1)  # [4, 128]
    per_dir = {
        0: (np.ascontiguousarray(Wx_f, dtype=bf), np.ascontiguousarray(Wh_f, dtype=bf),
            np.ascontiguousarray(b_f.reshape(4, -1), dtype=bf)),
        1: (np.ascontiguousarray(Wx_b, dtype=bf), np.ascontiguousarray(Wh_b, dtype=bf),
            np.ascontiguousarray(b_b.reshape(4, -1), dtype=bf)),
    }
    in_maps = []
    for c in range(8):
        d, q = c // 4, c % 4
        tok = inputs[BP * q:BP * (q + 1), :]          # [16, S]
        if d == 1:
            tok = tok[:, ::-1]
        tok = np.ascontiguousarray(tok.T.reshape(-1, 1), dtype=np.int32)  # [(t,b), 1]
        wx, wh, bias = per_dir[d]
        in_maps.append({
            "tok": tok, "emb": emb, "wx": wx, "wh": wh, "bias": bias,
            "eye16": eye16, "eye128": eye128, "sel4": sel4,
        })
    return in_maps


def run_cores(in_maps, S=512, V=32000, trace=False, tmpdir=None):
    nc = _get_program(S=S, V=V)
    kwargs = {}
    if trace:
        kwargs = dict(trace=True, tmpdir=tmpdir)
    res = bass_utils.run_bass_kernel_spmd(nc, in_maps, core_ids=list(range(8)), **kwargs)
    return res


def assemble(results, B=64, S=512):
    outputs = np.empty((B, S, 2 * H), dtype=np.float32)
    hidden = np.empty((B, 2 * H), dtype=np.float32)
    cell = np.empty((B, 2 * H), dtype=np.float32)
    for c in range(8):
        d, q = c // 4, c % 4
        hs = np.asarray(results[c]["hs"], dtype=np.float32)   # [S, 16, H]
        c_out = np.asarray(results[c]["c_out"], dtype=np.float32)
        rows = slice(BP * q, BP * (q + 1))
        if d == 0:
            outputs[rows, :, :H] = hs.transpose(1, 0, 2)
            hidden[rows, :H] = hs[-1]
            cell[rows, :H] = c_out
        else:
            outputs[rows, :, H:] = hs[::-1].transpose(1, 0, 2)
            hidden[rows, H:] = hs[-1]
            cell[rows, H:] = c_out
    return outputs, hidden, cell


def kernel(inputs, embed_table, Wx_f, Wh_f, b_f, Wx_b, Wh_b, b_b):
    B, S = inputs.shape
    in_maps = make_in_maps(inputs, embed_table, Wx_f, Wh_f, b_f, Wx_b, Wh_b, b_b)
    res = run_cores(in_maps, S=S, V=embed_table.shape[0])
    return assemble(res.results, B=B, S=S)
